# revision 17
# baseline (speedup 1.0000x reference)
"""ConvTranspose2d (kernel=stride=2) as GEMM + pixel-shuffle on 8 TRN2 cores.

Problem: x (8, 512, 64, 64) f32, weight (512, 256, 2, 2), bias (256,)
         -> out (8, 256, 128, 128) f32.

Sharding: data-parallel over batch N: core b handles batch b. Weight/bias
replicated. No collectives.

Per-core GEMM: out[(i,j,o), (h,w)] = sum_c wfold[c, (i,j,o)] * x[c, (h,w)]
  K = 512, M = 1024 = 4 taps x 256 o, N = 4096 pixels (8 chunks of 512).

Hybrid precision (the PE is the bottleneck; fp8 runs at bf16 rate unless
DoubleRow, and DoubleRow requires e4m3 whose 3 mantissa bits are too coarse
for the full GEMM):
  - channels 0..255: ONE DoubleRow matmul per (m-tile, chunk) contracting
    256 channels: lhsT = (w - 0.5) in e4m3 [128, 2, 128], rhs = x in e4m3
    [128, 2, 512]. Centering w halves its quantization noise; the exact
    compensation term 0.5 * colsum(x_e4m3) is computed on-device by a
    DoubleRow matmul with a constant-0.5 stationary (result replicated
    across all 128 output partitions) and added during the drain.
  - channels 256..511: two bf16-rate matmuls with fp16 weights and fp16 x
    (fp16 x kills that half's quantization noise; we are PE-bound, not
    DMA-bound, so the extra x bytes are free).
Per (m-tile, chunk): 3 matmuls (585+2*518 cycles) vs 4 bf16 (4*518):
0.78x PE time. Exact end-to-end error on the fixed inputs: 1.80e-2
(gate 2e-2; all-bf16 baseline 1.22e-2, full-DoubleRow 2.8e-2 fails).

Loop order is M-tile outer / half(4 chunks) / stage / chunk with the full x
resident in SBUF. Runs of 4 matmuls share one stationary tile; a
post-schedule pass drops the redundant InstLdweights the legalizer pairs
with each matmul, and ordering-only scheduler edges pin the matmul order.
PSUM ping-pongs 2 half-tiles of 4 banks so drains never stall the PE.

The pixel shuffle out[o, 2h+i, 2w+j] is folded into the PSUM->SBUF drain
(DVE/GPSIMD scalar_tensor_tensor: (psum + bias) + correction, strided fp16
write into per-(half, chunk) staging), so DRAM stores are contiguous.
Output is staged fp16 and upcast to fp32 on the host.
"""
import numpy as np
from contextlib import ExitStack

import concourse.tile as tile
from concourse import bacc, mybir
from concourse.bass_utils import run_bass_kernel_spmd
from concourse.tile import add_dep_helper
from concourse.alu_op_type import AluOpType

N_CORES = 8
IN_C, OUT_C, S = 512, 256, 2
H = W = 64
OH, OW = H * S, W * S          # 128, 128
N_FULL = H * W                 # 4096 pixels
NCH = 512                      # N-chunk (one PSUM bank)
N_CHUNKS = N_FULL // NCH       # 8
H_PER = NCH // W               # 8 input rows per chunk
YROWS = H_PER * S              # 16 output rows per chunk
M_FOLD = OUT_C * S * S         # 1024

_built = None


def _dedup_ldweights(nc):
    """Drop InstLdweights whose weights AP (and perf mode) equals the
    immediately preceding ldweights in the same block. The PE array retains
    loaded weights across matmuls, so a run of matmuls sharing one
    stationary tile needs only the first load. A dropped ldweights'
    semaphore waits/updates are merged into the immediately following
    matmul (same engine queue position, so gating semantics are
    unchanged)."""
    removed = 0
    for blk in nc.m.functions[0].blocks:
        last_key = None
        keep = []
        pending = None          # sync_info from a just-removed ldweights
        for inst in blk.instructions:
            tn = type(inst).__name__
            if tn == "InstLdweights":
                ap = inst.ins[0]
                key = (ap.memref, ap.offset,
                       tuple(tuple(d) for d in ap.ap), str(ap.dtype),
                       str(inst.perf_mode), str(inst.is_transpose))
                if key == last_key and not ap.regs_read():
                    si = inst.sync_info
                    if si is not None and (si.on_wait or si.on_update):
                        assert pending is None
                        pending = si
                    removed += 1
                    continue
                last_key = key
            elif pending is not None:
                assert tn == "InstMatmult", tn
                si = inst.sync_info
                if si is None:
                    inst.sync_info = mybir.SyncInfo(
                        on_wait=list(pending.on_wait),
                        on_update=list(pending.on_update))
                else:
                    si.on_wait[:0] = pending.on_wait
                    si.on_update.extend(pending.on_update)
                pending = None
            keep.append(inst)
        assert pending is None
        blk.instructions[:] = keep
    return removed


def _build(repeats: int = 1, unroll: int = 12, staggered: bool = True,
           dedup: bool = True, halves: bool = True):
    DR = mybir.MatmulPerfMode.DoubleRow
    f8e4 = mybir.dt.float8e4
    out_dt = mybir.dt.float16
    nc = bacc.Bacc("TRN2", debug=False, num_devices=N_CORES)
    # channels 0..255 as e4m3: [slot s][p] = channel s*128+p
    x4d = nc.dram_tensor("x4", [128, 2, N_FULL], f8e4, kind="ExternalInput")
    # channels 256..511 as fp16: [t][p] = channel 256+t*128+p
    x16d = nc.dram_tensor("x16", [2, 128, N_FULL], mybir.dt.float16,
                          kind="ExternalInput")
    w4d = nc.dram_tensor("w4", [128, 2, M_FOLD], f8e4, kind="ExternalInput")
    w16d = nc.dram_tensor("w16", [2, 128, M_FOLD], mybir.dt.float16,
                          kind="ExternalInput")
    h05d = nc.dram_tensor("h05", [128, 2, 128], f8e4, kind="ExternalInput")
    bd = nc.dram_tensor("b", [2, 128, 1], mybir.dt.float32,
                        kind="ExternalInput")
    od = nc.dram_tensor("out", [OUT_C, OH, OW], out_dt,
                        kind="ExternalOutput")

    with tile.TileContext(nc) as tc, ExitStack() as ctx:
        wpool = ctx.enter_context(tc.tile_pool(name="wp", bufs=1))
        bpool = ctx.enter_context(tc.tile_pool(name="bp", bufs=1))
        xpool = ctx.enter_context(tc.tile_pool(name="xp", bufs=2))
        cpool = ctx.enter_context(tc.tile_pool(name="cp", bufs=1))
        spool = ctx.enter_context(tc.tile_pool(name="sp", bufs=1))
        ppool = ctx.enter_context(tc.tile_pool(name="pp", bufs=8,
                                               space="PSUM"))

        # Resident weights/constants.
        wt4 = wpool.tile([128, 2 * M_FOLD], f8e4, tag="w4")
        nc.sync.dma_start(wt4[:],
                          w4d.ap().rearrange("p two m -> p (two m)"))
        wt16 = []
        for t in range(2):
            w = wpool.tile([128, M_FOLD], mybir.dt.float16, tag=f"w16_{t}",
                           name=f"wt16_{t}")
            nc.sync.dma_start(w[:], w16d.ap()[t])
            wt16.append(w)
        h05 = wpool.tile([128, 2 * 128], f8e4, tag="h05")
        nc.sync.dma_start(h05[:], h05d.ap().rearrange("p two m -> p (two m)"))
        bts = []
        for g in range(2):
            t = bpool.tile([128, 1], mybir.dt.float32, tag=f"bias{g}",
                           name=f"bias_{g}")
            nc.sync.dma_start(t[:], bd.ap()[g])
            bts.append(t)
        h05v = h05[:].rearrange("p (two m) -> p two m", two=2)

        def body(prev_mm=None):
            # Full x for this iteration, double-buffered across iterations.
            xt4 = xpool.tile([128, 2 * N_FULL], f8e4, tag="x4")
            nc.sync.dma_start(xt4[:],
                              x4d.ap().rearrange("p two n -> p (two n)"))
            xt16 = []
            for t in range(2):
                xt = xpool.tile([128, N_FULL], mybir.dt.float16,
                                tag=f"x16_{t}", name=f"xt16_{t}")
                nc.sync.dma_start(xt[:], x16d.ap()[t])
                xt16.append(xt)
            x4v = xt4[:].rearrange("p (two n) -> p two n", two=2)

            def chain(mm):
                nonlocal prev_mm
                if prev_mm is not None:
                    add_dep_helper(mm.ins, prev_mm.ins, False, "mm order")
                prev_mm = mm

            # Correction phase: corr[c] = 0.5 * colsum(x_e4m3 chunk c),
            # replicated over the 128 output partitions by the matmul.
            cbs = []
            for c in range(N_CHUNKS):
                cp = ppool.tile([128, NCH], mybir.dt.float32, tag="ps",
                                name=f"cps_{c}")
                chain(nc.tensor.matmul(
                    cp[:], h05v, x4v[:, :, c * NCH:(c + 1) * NCH],
                    start=True, stop=True, perf_mode=DR))
                cb = cpool.tile([128, NCH], out_dt, tag=f"cb{c}",
                                name=f"cb_{c}")
                # Alternate engines so bank releases pace the main matmuls.
                if c % 2 == 0:
                    nc.vector.tensor_copy(cb[:], cp[:])
                else:
                    nc.scalar.copy(cb[:], cp[:])
                cbs.append(cb)

            # Staging tiles: one per (o-half g, chunk c), [128, 16*128] fp16.
            sts = [[spool.tile([128, YROWS * OW], out_dt, tag=f"s{g}_{c}",
                               name=f"st_g{g}_c{c}")
                    for c in range(N_CHUNKS)] for g in range(2)]

            for g in range(2):
                for ij in range(4):
                    i, j = ij // 2, ij % 2
                    m0 = ij * OUT_C + g * 128
                    # Stage-major runs of 8 share one stationary; one
                    # LDWEIGHTS per stage per m-tile. DR LDWEIGHTS (256
                    # cols, no FWL) is only half-hidden, so fewer is
                    # better; all-DVE drains (~0.4us each) release banks
                    # well within the 5.4us m-tile window.
                    for half in ([0, 1] if halves else [0]):
                        cs = (range(half * 4, half * 4 + 4) if halves
                              else range(N_CHUNKS))
                        pts = {c: ppool.tile([128, NCH], mybir.dt.float32,
                                             tag="ps", name=f"ps_c{c}")
                               for c in cs}
                        # Stage-major: runs of 4 matmuls share a stationary.
                        for c in cs:
                            chain(nc.tensor.matmul(
                                pts[c][:],
                                wt4[:].rearrange("p (two m) -> p two m",
                                                 two=2)[:, :, m0:m0 + 128],
                                x4v[:, :, c * NCH:(c + 1) * NCH],
                                start=True, stop=False, perf_mode=DR))
                        for t in range(2):
                            for c in cs:
                                chain(nc.tensor.matmul(
                                    pts[c][:],
                                    wt16[t][:, m0:m0 + 128],
                                    xt16[t][:, c * NCH:(c + 1) * NCH],
                                    start=False, stop=(t == 1)))
                        # Drain: (psum + bias) + corr -> staging, shuffled.
                        for c in cs:
                            s5 = sts[g][c][:].rearrange(
                                "p (h i w j) -> p h i w j", i=S, w=W, j=S)
                            src = pts[c][:].rearrange("p (h w) -> p h w",
                                                      w=W)
                            cbv = cbs[c][:].rearrange("p (h w) -> p h w",
                                                      w=W)
                            dst = s5[:, :, i, :, j]
                            # GPSIMD cannot read PSUM; ACT has no
                            # tensor_tensor -> all drains ride DVE (64 x
                            # ~0.4us fits well under the PE-bound body).
                            nc.vector.scalar_tensor_tensor(
                                dst, src, bts[g][:, 0:1], cbv,
                                AluOpType.add, AluOpType.add)
                # All 4 taps of half g drained: store the 8 chunks.
                for c in range(N_CHUNKS):
                    od3 = od.ap()[g * 128:(g + 1) * 128,
                                  c * YROWS:(c + 1) * YROWS, :]
                    eng = nc.scalar if c % 2 == 0 else nc.gpsimd
                    eng.dma_start(
                        od3, sts[g][c][:].rearrange("p (y x) -> p y x",
                                                    x=OW))
            return prev_mm

        full, rem = divmod(repeats, unroll)
        if full >= 2:
            with tc.For_i(0, full, 1, staggered_reset=staggered):
                prev = None
                for _ in range(unroll):
                    prev = body(prev)
        else:
            rem = repeats
        prev = None
        for _ in range(rem):
            prev = body(prev)

    if dedup:
        _dedup_ldweights(nc)
    nc.compile()
    return nc


def prep_inputs(x, weight, bias):
    import ml_dtypes
    x = np.asarray(x, dtype=np.float32)
    weight = np.asarray(weight, dtype=np.float32)
    bias = np.asarray(bias, dtype=np.float32)
    # [c, o, i, j] -> [c, (i j o)]: an M-tile of 128 is one o-half of one
    # (i, j) tap, so the GEMM output partition dim is o (bias per partition,
    # contiguous DRAM rows per o).
    wfold = np.ascontiguousarray(
        weight.transpose(0, 2, 3, 1).reshape(IN_C, M_FOLD))
    w4 = np.ascontiguousarray(
        (wfold[:256] - 0.5).reshape(2, 128, M_FOLD).transpose(1, 0, 2)
        .astype(ml_dtypes.float8_e4m3))
    w16 = np.ascontiguousarray(
        wfold[256:].reshape(2, 128, M_FOLD).astype(np.float16))
    h05 = np.full((128, 2, 128), 0.5, dtype=ml_dtypes.float8_e4m3)
    bfold = np.ascontiguousarray(bias.reshape(2, 128, 1))
    out = []
    for b in range(N_CORES):
        xb = x[b].reshape(IN_C, N_FULL)
        x4 = np.ascontiguousarray(
            xb[:256].reshape(2, 128, N_FULL).transpose(1, 0, 2)
            .astype(ml_dtypes.float8_e4m3))
        x16 = np.ascontiguousarray(
            xb[256:].reshape(2, 128, N_FULL).astype(np.float16))
        out.append({"x4": x4, "x16": x16, "w4": w4, "w16": w16,
                    "h05": h05, "b": bfold})
    return out


def kernel(x: np.ndarray, weight: np.ndarray, bias: np.ndarray) -> np.ndarray:
    global _built
    if _built is None:
        _built = _build()
    nc = _built
    in_maps = prep_inputs(x, weight, bias)
    res = run_bass_kernel_spmd(nc, in_maps, core_ids=list(range(N_CORES)))
    out = np.stack([res.results[b]["out"] for b in range(N_CORES)], axis=0)
    return np.ascontiguousarray(out.astype(np.float32))
